# revision 10
# baseline (speedup 1.0000x reference)
import sys

sys.path.insert(0, "/opt/trn_rl_repo")

from contextlib import ExitStack

import numpy as np

import concourse.bacc as bacc
import concourse.tile as tile
from concourse import mybir
from concourse.bass_utils import run_bass_kernel_spmd

# problem constants
N, E, M, D = 10000, 5000, 200000, 128
K, DD = 8, 16
NITER, NFOLD, EPS = 2, 20, 1e-12
C = 8  # cores

ES, NCH1 = 640, 5  # edge rows per core (625 valid; range ownership)
NS, NCH2 = 1280, 10  # node rows per core (1250 valid; node%8 ownership)
EV, NV = 625, 1250
NPAD = 10112  # 79 * 128
KT = NPAD // 128
MAXSLOT = 16  # cap on token tiles per fold (PSUM budget)

_cache = {}
TRACE = False
LAST_EXEC_NS = None


def _wrap_idx(idx):
    # token i -> [i%16 partition (replicated across 8 q7 cores), i//16 col]
    n = idx.shape[0]
    w = idx.reshape(n // 16, 16).T.astype(np.int16)
    return np.tile(w, (8, 1)).copy()


def _prep_stage(src_idx, dst_core, dst_local, fold_len, nch):
    """Per-fold adaptive grid, uniform across cores.

    Returns:
      grid: list over folds of list of (chunk, ntiles)
      zidx: per-core [128, sum_f ntok_f // 16] int16 (wrapped)
      st:   per-core [TT, 128, 128] bf16  (dst x tok one-hots)
      s:    per-core [TT, 128, 128] bf16  (tok x dst)
    """
    import ml_dtypes

    per_fc = []  # [f][c] -> (si, chunk, row) sorted by chunk
    grid = []
    for f in range(NFOLD):
        lo, hi = f * fold_len, (f + 1) * fold_len
        si_f = src_idx[lo:hi]
        co_f = dst_core[lo:hi]
        dl_f = dst_local[lo:hi]
        row_f, ch_f = dl_f % 128, dl_f // 128
        maxtiles = np.zeros(nch, np.int64)
        fc = []
        for c in range(C):
            m = co_f == c
            si, ch, row = si_f[m], ch_f[m], row_f[m]
            order = np.argsort(ch, kind="stable")
            si, ch, row = si[order], ch[order], row[order]
            cnt = np.bincount(ch, minlength=nch)
            maxtiles = np.maximum(maxtiles, -(-cnt // 128))
            fc.append((si, ch, row))
        g = [(k, int(maxtiles[k])) for k in range(nch) if maxtiles[k] > 0]
        assert sum(t for _, t in g) <= MAXSLOT, f"fold {f}: too many tiles {g}"
        grid.append(g)
        per_fc.append(fc)

    tt = sum(sum(t for _, t in g) for g in grid)
    zidx_all, st_all, s_all = [], [], []
    for c in range(C):
        zcols = []
        st_c = np.zeros((tt, 128, 128), np.float32)
        tbase = 0
        for f in range(NFOLD):
            si, ch, row = per_fc[f][c]
            ntok_f = 128 * sum(t for _, t in grid[f])
            zidx_f = np.zeros(ntok_f, np.int64)
            base = 0
            for k, ntile in grid[f]:
                m = ch == k
                cnt = int(m.sum())
                zidx_f[base : base + cnt] = si[m]
                rows_k = row[m]
                for j in range(ntile):
                    a, b = j * 128, min((j + 1) * 128, cnt)
                    if a < cnt:
                        t = np.arange(a, b)
                        st_c[tbase + j, rows_k[t], t - a] = 1.0
                base += ntile * 128
                tbase += ntile
            zcols.append(_wrap_idx(zidx_f.astype(np.int16)))
        zidx_all.append(np.concatenate(zcols, axis=1))
        st_all.append(st_c.astype(ml_dtypes.bfloat16))
        s_all.append(st_c.transpose(0, 2, 1).copy().astype(ml_dtypes.bfloat16))
    return grid, zidx_all, st_all, s_all


def _build_program(grid1, zcols1, tt1, grid2, zcols2, tt2):
    nc = bacc.Bacc("TRN2", target_bir_lowering=False, debug=False, num_devices=C)
    f32, bf16, i16 = mybir.dt.float32, mybir.dt.bfloat16, mybir.dt.int16

    xp = nc.declare_dram_parameter("xp", [NPAD, D], f32, isOutput=False)
    adj = nc.declare_dram_parameter("adj", [ES, NPAD], f32, isOutput=False)
    xn = nc.declare_dram_parameter("xn", [NS, D], f32, isOutput=False)
    ident = nc.declare_dram_parameter("ident", [128, 128], f32, isOutput=False)
    zi1 = nc.declare_dram_parameter("zi1", [128, zcols1], i16, isOutput=False)
    st1 = nc.declare_dram_parameter("st1", [tt1, 128, 128], bf16, isOutput=False)
    s1 = nc.declare_dram_parameter("s1", [tt1, 128, 128], bf16, isOutput=False)
    zi2 = nc.declare_dram_parameter("zi2", [128, zcols2], i16, isOutput=False)
    st2 = nc.declare_dram_parameter("st2", [tt2, 128, 128], bf16, isOutput=False)
    s2 = nc.declare_dram_parameter("s2", [tt2, 128, 128], bf16, isOutput=False)
    ue_out = nc.declare_dram_parameter("ue_out", [ES, D], f32, isOutput=True)
    un_out = nc.declare_dram_parameter("un_out", [NS, D], f32, isOutput=True)

    xcap_bf = nc.dram_tensor("xcap_bf", [N, D], bf16)
    ue_bounce = nc.dram_tensor("ue_bounce", [ES, D], bf16)
    ue_full = nc.dram_tensor("ue_full", [C * ES, D], bf16, addr_space="Shared")

    def capnorm_aps(pool, aps):
        for a in aps:
            w = a.shape[1]
            sq = pool.tile([128, w], f32, tag="cn_sq", name="cn_sq")
            nc.vector.tensor_tensor(out=sq[:], in0=a, in1=a, op=mybir.AluOpType.mult)
            ss = pool.tile([128, w // DD], f32, tag="cn_ss", name="cn_ss")
            nc.vector.tensor_reduce(
                out=ss[:],
                in_=sq[:].rearrange("p (c d) -> p c d", d=DD),
                axis=mybir.AxisListType.X,
                op=mybir.AluOpType.add,
            )
            nc.scalar.activation(
                out=ss[:], in_=ss[:], func=mybir.ActivationFunctionType.Sqrt
            )
            nc.vector.tensor_scalar_max(out=ss[:], in0=ss[:], scalar1=float(EPS))
            rec = pool.tile([128, w // DD], f32, tag="cn_rec", name="cn_rec")
            nc.vector.reciprocal(out=rec[:], in_=ss[:])
            nc.vector.tensor_tensor(
                out=a.rearrange("p (c d) -> p c d", d=DD),
                in0=a.rearrange("p (c d) -> p c d", d=DD),
                in1=rec[:][:, :, None].to_broadcast([128, w // DD, DD]),
                op=mybir.AluOpType.mult,
            )

    def routing_stage(zpool, wpool, spool, pspool, dpool,
                      grid, zi, st, s, ztable, u_f32, u_bf):
        nslots = [sum(t for _, t in g) for g in grid]
        idx_tiles, z_tiles = [], []
        off = 0
        for f in range(NFOLD):
            ntok = nslots[f] * 128
            it = zpool.tile([128, ntok // 16], i16, tag=f"idx{f}", name=f"idx{f}")
            nc.sync.dma_start(it[:], zi[:, off : off + ntok // 16])
            off += ntok // 16
            idx_tiles.append(it)
            z_tiles.append(
                zpool.tile([128, nslots[f], D], bf16, tag=f"z{f}", name=f"z{f}")
            )
        fold_tbase = np.cumsum([0] + [s_ for s_ in nslots])
        for itr in range(NITER):
            for f in range(NFOLD):
                nslot = nslots[f]
                ntok = nslot * 128
                z = z_tiles[f]
                if itr == 0:
                    nc.gpsimd.dma_gather(
                        z[:], ztable[:], idx_tiles[f][:], ntok, ntok, D,
                        single_packet=False,
                    )
                tb = int(fold_tbase[f])
                ug = pspool.tile([128, MAXSLOT * D], f32, tag="ug", name="ug")
                jt = 0
                for k, ntile in grid[f]:
                    for _ in range(ntile):
                        stt = spool.tile([128, 128], bf16, tag="st_stream",
                                         name="st_stream")
                        nc.sync.dma_start(stt[:], st[tb + jt])
                        nc.tensor.matmul(
                            out=ug[:, jt * D : (jt + 1) * D],
                            lhsT=stt[:],
                            rhs=u_bf[k][:],
                            start=True,
                            stop=True,
                        )
                        jt += 1
                prod = wpool.tile([128, MAXSLOT * D], f32, tag="prod", name="prod")
                nc.vector.tensor_tensor(
                    out=prod[:, : nslot * D],
                    in0=ug[:, : nslot * D],
                    in1=z[:].rearrange("p s d -> p (s d)"),
                    op=mybir.AluOpType.mult,
                )
                caps = wpool.tile([128, MAXSLOT * K], f32, tag="caps", name="caps")
                nc.vector.tensor_reduce(
                    out=caps[:, : nslot * K],
                    in_=prod[:, : nslot * D].rearrange(
                        "p (s c d) -> p (s c) d", c=K, d=DD
                    ),
                    axis=mybir.AxisListType.X,
                    op=mybir.AluOpType.add,
                )
                ex = wpool.tile([128, MAXSLOT * K], f32, tag="ex", name="ex")
                nc.scalar.activation(
                    out=ex[:, : nslot * K], in_=caps[:, : nslot * K],
                    func=mybir.ActivationFunctionType.Exp,
                )
                esum = wpool.tile([128, MAXSLOT], f32, tag="esum", name="esum")
                nc.vector.tensor_reduce(
                    out=esum[:, :nslot],
                    in_=ex[:, : nslot * K].rearrange("p (s c) -> p s c", c=K),
                    axis=mybir.AxisListType.X,
                    op=mybir.AluOpType.add,
                )
                nc.vector.reciprocal(out=esum[:, :nslot], in_=esum[:, :nslot])
                p = wpool.tile([128, MAXSLOT * K], f32, tag="p", name="p")
                nc.vector.tensor_tensor(
                    out=p[:, : nslot * K].rearrange("p (s c) -> p s c", c=K),
                    in0=ex[:, : nslot * K].rearrange("p (s c) -> p s c", c=K),
                    in1=esum[:, :nslot][:, :, None].to_broadcast([128, nslot, K]),
                    op=mybir.AluOpType.mult,
                )
                scat = wpool.tile([128, MAXSLOT * D], bf16, tag="scat", name="scat")
                nc.vector.tensor_tensor(
                    out=scat[:, : nslot * D].rearrange(
                        "p (s c d) -> p s c d", c=K, d=DD
                    ),
                    in0=z[:].rearrange("p s (c d) -> p s c d", c=K, d=DD),
                    in1=p[:, : nslot * K].rearrange("p (s c) -> p s c", c=K)
                    [:, :, :, None].to_broadcast([128, nslot, K, DD]),
                    op=mybir.AluOpType.mult,
                )
                jt = 0
                for gi, (k, ntile) in enumerate(grid[f]):
                    dl = dpool.tile([128, D], f32, tag=f"delta{gi % 2}",
                                    name=f"delta{gi % 2}")
                    for j in range(ntile):
                        stile = spool.tile([128, 128], bf16, tag="s_stream",
                                           name="s_stream")
                        nc.sync.dma_start(stile[:], s[tb + jt])
                        nc.tensor.matmul(
                            out=dl[:],
                            lhsT=stile[:],
                            rhs=scat[:, jt * D : (jt + 1) * D],
                            start=(j == 0),
                            stop=(j == ntile - 1),
                        )
                        jt += 1
                    nc.vector.tensor_tensor(
                        out=u_f32[k][:], in0=u_f32[k][:], in1=dl[:],
                        op=mybir.AluOpType.add,
                    )
                    nc.vector.tensor_copy(out=u_bf[k][:], in_=u_f32[k][:])
            capnorm_aps(wpool, [t[:] for t in u_f32])
            for t, tb_ in zip(u_f32, u_bf):
                nc.vector.tensor_copy(out=tb_[:], in_=t[:])

    with tile.TileContext(nc) as tc, ExitStack() as top:
        upool = top.enter_context(tc.tile_pool(name="upool", bufs=1))
        u1_f32 = [upool.tile([128, D], f32, tag=f"u1f{k}", name=f"u1f{k}") for k in range(NCH1)]
        u1_bf = [upool.tile([128, D], bf16, tag=f"u1b{k}", name=f"u1b{k}") for k in range(NCH1)]
        u2_f32 = [upool.tile([128, D], f32, tag=f"u2f{k}", name=f"u2f{k}") for k in range(NCH2)]
        u2_bf = [upool.tile([128, D], bf16, tag=f"u2b{k}", name=f"u2b{k}") for k in range(NCH2)]
        idsb = upool.tile([128, 128], f32, tag="ident", name="identsb")
        nc.sync.dma_start(idsb[:], ident[:])

        # ---- phase 0: capnorm(x); edge_emb = capnorm(adj @ x_cap) ----
        with (
            tc.tile_pool(name="p0", bufs=1) as p0,
            tc.tile_pool(name="p0a", bufs=2) as p0a,
            tc.tile_pool(name="ps0", bufs=2, space="PSUM") as ps0,
        ):
            xsb = p0.tile([128, KT * D], f32, tag="xsb", name="xsb")
            nc.sync.dma_start(
                xsb[:].rearrange("p (t d) -> p t d", d=D),
                xp[:].rearrange("(t p) d -> p t d", p=128),
            )
            capnorm_aps(
                p0a,
                [xsb[:, i * 2048 : (i + 1) * 2048] for i in range(4)]
                + [xsb[:, 8192 : KT * D]],
            )
            xbf = p0.tile([128, KT * D], bf16, tag="xbf", name="xbf")
            nc.vector.tensor_copy(out=xbf[:], in_=xsb[:])
            nc.sync.dma_start(
                xcap_bf[: 78 * 128, :].rearrange("(t p) d -> p t d", p=128),
                xbf[:, : 78 * D].rearrange("p (t d) -> p t d", d=D),
            )
            nc.sync.dma_start(xcap_bf[78 * 128 : N, :], xbf[0:16, 78 * D : 79 * D])

            for g in range(NCH1):
                ee = ps0.tile([128, D], f32, tag="ee", name="ee")
                bounds = [0, 20, 40, 60, KT]
                for sub in range(4):
                    k0, k1 = bounds[sub], bounds[sub + 1]
                    asl = p0a.tile([128, (k1 - k0) * 128], f32, tag="aslab", name="aslab")
                    nc.sync.dma_start(
                        asl[:], adj[g * 128 : (g + 1) * 128, k0 * 128 : k1 * 128]
                    )
                    for k in range(k0, k1):
                        att = ps0.tile([128, 128], f32, tag="att", name="att")
                        nc.tensor.transpose(
                            out=att[:],
                            in_=asl[:, (k - k0) * 128 : (k - k0 + 1) * 128],
                            identity=idsb[:],
                        )
                        ats = p0a.tile([128, 128], f32, tag="ats", name="ats")
                        nc.vector.tensor_copy(out=ats[:], in_=att[:])
                        nc.tensor.matmul(
                            out=ee[:],
                            lhsT=ats[:],
                            rhs=xsb[:, k * D : (k + 1) * D],
                            start=(k == 0),
                            stop=(k == KT - 1),
                        )
                nc.vector.tensor_copy(out=u1_f32[g][:], in_=ee[:])
            capnorm_aps(p0a, [t[:] for t in u1_f32])
            for t, tb in zip(u1_f32, u1_bf):
                nc.vector.tensor_copy(out=tb[:], in_=t[:])

        # ---- stage 1 ----
        with (
            tc.tile_pool(name="r1z", bufs=1) as r1z,
            tc.tile_pool(name="r1w", bufs=2) as r1w,
            tc.tile_pool(name="r1s", bufs=30) as r1s,
            tc.tile_pool(name="ps1u", bufs=1, space="PSUM") as ps1u,
            tc.tile_pool(name="ps1d", bufs=1, space="PSUM") as ps1d,
        ):
            routing_stage(r1z, r1w, r1s, ps1u, ps1d,
                          grid1, zi1, st1, s1, xcap_bf, u1_f32, u1_bf)
            for k in range(NCH1):
                nc.sync.dma_start(ue_out[k * 128 : (k + 1) * 128, :], u1_f32[k][:])
                nc.sync.dma_start(ue_bounce[k * 128 : (k + 1) * 128, :], u1_bf[k][:])
            nc.gpsimd.collective_compute(
                "AllGather",
                mybir.AluOpType.bypass,
                replica_groups=[list(range(C))],
                ins=[ue_bounce[:]],
                outs=[ue_full[:]],
            )

        # ---- stage 2 ----
        with (
            tc.tile_pool(name="r2z", bufs=1) as r2z,
            tc.tile_pool(name="r2w", bufs=2) as r2w,
            tc.tile_pool(name="r2s", bufs=30) as r2s,
            tc.tile_pool(name="ps2u", bufs=1, space="PSUM") as ps2u,
            tc.tile_pool(name="ps2d", bufs=1, space="PSUM") as ps2d,
        ):
            for k in range(NCH2):
                nc.sync.dma_start(u2_f32[k][:], xn[k * 128 : (k + 1) * 128, :])
            capnorm_aps(r2w, [t[:] for t in u2_f32])
            for t, tb in zip(u2_f32, u2_bf):
                nc.vector.tensor_copy(out=tb[:], in_=t[:])
            routing_stage(r2z, r2w, r2s, ps2u, ps2d,
                          grid2, zi2, st2, s2, ue_full, u2_f32, u2_bf)
            for k in range(NCH2):
                nc.sync.dma_start(un_out[k * 128 : (k + 1) * 128, :], u2_f32[k][:])

    nc.compile()
    return nc


def _prepare(x, adjacency, edge_node):
    x = np.asarray(x, np.float32)
    adjacency = np.asarray(adjacency, np.float32)
    edge_node = np.asarray(edge_node, np.int32)

    edges_g = edge_node[0].astype(np.int64)
    nodes_g = edge_node[1].astype(np.int64)
    order = np.argsort(nodes_g, kind="stable")
    node_ns = nodes_g[order]
    edge_ns = edges_g[order]
    fold_len = M // NFOLD

    # stage 1: src=node ids, dst=edge ids (range ownership)
    grid1, zidx1, st1_h, s1_h = _prep_stage(
        nodes_g, edges_g // EV, edges_g % EV, fold_len, NCH1
    )
    # stage 2: src=edge rows (padded global), dst=node ids (mod-8 ownership)
    edge_pad = (edge_ns // EV) * ES + (edge_ns % EV)
    grid2, zidx2, st2_h, s2_h = _prep_stage(
        edge_pad, node_ns % C, node_ns // C, fold_len, NCH2
    )

    xp = np.zeros((NPAD, D), np.float32)
    xp[:N] = x
    ident = np.eye(128, dtype=np.float32)

    key = "nc"
    if key not in _cache:
        _cache[key] = _build_program(
            grid1, zidx1[0].shape[1], st1_h[0].shape[0],
            grid2, zidx2[0].shape[1], st2_h[0].shape[0],
        )
    nc = _cache[key]

    in_maps = []
    for c in range(C):
        adj_c = np.zeros((ES, NPAD), np.float32)
        adj_c[:EV, :N] = adjacency[c * EV : (c + 1) * EV]
        xn_c = np.zeros((NS, D), np.float32)
        xn_c[:NV] = x[c::C]
        in_maps.append(
            {
                "xp": xp, "adj": adj_c, "xn": xn_c, "ident": ident,
                "zi1": zidx1[c], "st1": st1_h[c], "s1": s1_h[c],
                "zi2": zidx2[c], "st2": st2_h[c], "s2": s2_h[c],
            }
        )

    return nc, in_maps


def kernel(x, adjacency, edge_node):
    nc, in_maps = _prepare(x, adjacency, edge_node)
    res = run_bass_kernel_spmd(nc, in_maps, list(range(C)))
    ue = np.concatenate([res.results[c]["ue_out"][:EV] for c in range(C)], axis=0)
    un = np.zeros((N, D), np.float32)
    for c in range(C):
        un[c::C] = res.results[c]["un_out"][:NV]
    return un, ue
